# revision 53
# baseline (speedup 1.0000x reference)
"""Trainium2 Bass kernel for nn_GNN_82781199663565 (gnn_message_passing).

Computation (see reference):
  du = relu(BN(einsum(h_att[1]*xp, Wu)))   # [B, 40, H, W]
  dl = relu(BN(einsum(h_att[2]*xp, Wl)))   # [B, 20, H, W]
  p_new[0]   = 0.5*(h_nodes[0] + p_nodes[0])
  p_new[1:5] = 0.5*(p_nodes[1:5] + du4)    # du reshaped to [4, B, 10, H, W]
  p_new[5:7] = 0.5*(p_nodes[5:7] + dl2)
(f_nodes, h_att[0], h_nodes[1:] are unused.)

Strategy v2: data-parallel over H (32 rows per core, 8 cores), bf16 streams.
 - All big tensors (xp, attention, p_nodes residuals, outputs) travel as
   bf16: halves HBM traffic and runs the PE at full bf16 rate (fp32 matmul
   is ~4x slower).  End-to-end rel-err ~4e-3 (gate 2e-2).
 - One fused matmul z = Wcat.T @ xp per 512-col window, both batch images
   stacked on partitions (b0 -> 0:64, b1 -> 64:128).
 - Attention applied after the conv via a host-replicated bf16 [128, n]
   array; one DVE op computes y = z*a and accumulates sum(y); ACT squares
   for sum(y^2).
 - Sync-BN stats via ncfw AllReduce (8 cores, 1 KB).  No warm-up collective
   (a warm-up queues ahead on the FIFO CC stream and delays the real op);
   the pn/p0 prefetches are ordered after the cc_in store completes so HBM
   contention cannot delay the collective doorbell.  (A hand-rolled SWDGE
   remote-DMA mailbox was tried and reverted: under this runtime the remote
   sends take ~4 ms to deliver.)
 - p_nodes / h_nodes residuals are pre-halved host-side; phase 3 is
   d = relu(s'*y + t') on ACT and out = pnh + d on DVE, stores in bf16.
"""
import sys
sys.path.insert(0, '/opt/trn_rl_repo')

import numpy as np
import ml_dtypes

BF16 = ml_dtypes.bfloat16

N_CORES = 8
B, C, HID, H, W = 2, 256, 10, 256, 256
EPS = 1e-5
HS = H // N_CORES            # 32 H-rows per core
SPB = HS * W                 # spatial elems per batch image per core: 8192
M = 60                       # real output channels (40 u + 20 l)
MP = 64                      # padded to 64 -> groups tile partitions exactly
PP = 128
NB = 1024                    # matmul block (bf16 rhs max; 2 PSUM banks)
NQ = 2048                    # phase-3 window
NTOT = float(B * H * W)      # BN stat count: 131072
XN = 4096                    # superstep spatial columns (1 MiB bf16 loads)
QS = SPB // XN               # 2 supersteps

# packed fp32 constants column offsets: foldW, bcW, gamma, beta
C_FOLD = 0
C_BC = C_FOLD + M
C_GB = C_BC + PP
CW = C_GB + 2

_built = None


def _build():
    import concourse.bass as bass
    import concourse.tile as tile
    from concourse import mybir
    import bass_rust

    f32 = mybir.dt.float32
    bf16 = mybir.dt.bfloat16
    Alu = mybir.AluOpType
    Act = mybir.ActivationFunctionType

    nc = bass.Bass("TRN2", target_bir_lowering=False, debug=False,
                   num_devices=N_CORES, enable_partition_id=False)

    xp_d = nc.dram_tensor("xp", [C, B * SPB], bf16, kind="ExternalInput").ap()
    attb_d = nc.dram_tensor("attb", [PP, SPB], bf16, kind="ExternalInput").ap()
    pnh_d = nc.dram_tensor("pnh", [PP, SPB], bf16, kind="ExternalInput").ap()
    p0h_d = nc.dram_tensor("p0h", [128, 1280], bf16, kind="ExternalInput").ap()
    h0h_d = nc.dram_tensor("h0h", [128, 1280], bf16, kind="ExternalInput").ap()
    cpack_d = nc.dram_tensor("cpack", [128, CW], f32, kind="ExternalInput").ap()
    wtb_d = nc.dram_tensor("wtb", [128, 256], bf16, kind="ExternalInput").ap()

    out_d = nc.dram_tensor("out_main", [PP, SPB], bf16, kind="ExternalOutput").ap()
    out0_d = nc.dram_tensor("out0", [128, 1280], bf16, kind="ExternalOutput").ap()

    def pe_anchor(psum_tile, cp):
        # tiny matmul reading cp (seen by PE) writing one psum element:
        # absorbs the psum slot-release wait so real matmuls carry <=1 wait
        nc.tensor.matmul(psum_tile[0:1, 0:1], cp[0:1, 0:1], cp[0:1, 0:1],
                         start=True, stop=True, skip_group_check=True)

    with tile.TileContext(nc) as tc:
        with (
            tc.tile_pool(name="consts", bufs=1) as cpool,
            tc.tile_pool(name="attp", bufs=2) as attp,
            tc.tile_pool(name="xin", bufs=2) as xin,
            tc.tile_pool(name="ybuf", bufs=1) as ybuf,
            tc.tile_pool(name="sq", bufs=2) as sqp,
            tc.tile_pool(name="small", bufs=1) as sm,
            tc.tile_pool(name="pnl", bufs=2) as pnl,
            tc.tile_pool(name="p0l", bufs=1) as p0l,
            tc.tile_pool(name="obuf", bufs=2) as obuf,
            tc.tile_pool(name="zp", bufs=3, space="PSUM") as zp,
            tc.tile_pool(name="stp", bufs=1, space="PSUM") as stp,
            tc.tile_pool(name="dram", bufs=1, space="DRAM") as dr,
        ):
            # issue the first superstep's xp loads before anything else so
            # the HBM stream starts the moment the preamble ends
            xq0 = {}
            for b in range(B):
                for c in range(2):
                    t = xin.tile([128, XN], bf16, tag=f"x{b}{c}",
                                 name=f"x{b}{c}_0")
                    lo = b * SPB
                    nc.sync.dma_start(
                        t[:], xp_d[c * 128:(c + 1) * 128, lo:lo + XN])
                    xq0[(b, c)] = t
            abt0 = attp.tile([PP, XN], bf16, tag="attb", name="attb_0")
            nc.sync.dma_start(abt0[:], attb_d[:, 0:XN])

            cp = cpool.tile([128, CW], f32)
            nc.sync.dma_start(cp[:], cpack_d[:])
            wt = cpool.tile([128, 256], bf16, tag="wt")
            nc.sync.dma_start(wt[:], wtb_d[:])
            L4t = wt[0:4, 128:256]          # att-broadcast lhsT
            foldWt = cp[0:PP, C_FOLD:C_FOLD + M]
            bcWt = cp[0:M, C_BC:C_BC + PP]
            gam = cp[0:M, C_GB:C_GB + 1]      # 0.5*gamma (u|l)
            bet = cp[0:M, C_GB + 1:C_GB + 2]  # 0.5*beta

            y_full = ybuf.tile([PP, SPB], bf16)
            s1t = sm.tile([PP, (SPB // NB)], f32, tag="s1t")
            s2t = sm.tile([PP, (SPB // NB)], f32, tag="s2t")
            st = sm.tile([PP, 2], f32, tag="st")     # local BN partial sums

            # ---- PE warm-up: bf16 dummy matmuls trip the HAM toward the
            # 2.4 GHz state before the first xp tile lands ----
            wz = zp.tile([PP, NB], f32, tag="z", name="warm_z")
            for _ in range(16):
                nc.tensor.matmul(wz[0:128, 0:128], wt[:, 0:128], wt[:, 0:128],
                                 start=True, stop=True, skip_group_check=True)

            # ---- phase 1: stream xp, matmul, y = z*a, accumulate sums ----
            for qs in range(QS):
                if qs == 0:
                    xq, abt = xq0, abt0
                else:
                    xq = {}
                    for b in range(B):
                        for c in range(2):
                            t = xin.tile([128, XN], bf16, tag=f"x{b}{c}",
                                         name=f"x{b}{c}_{qs}")
                            lo = b * SPB + qs * XN
                            xdma = nc.sync.dma_start(
                                t[:], xp_d[c * 128:(c + 1) * 128, lo:lo + XN])
                            if qs == QS - 1 and b == B - 1 and c == 1:
                                last_xdma = xdma
                            xq[(b, c)] = t
                    abt = attp.tile([PP, XN], bf16, tag="attb",
                                    name=f"attb_{qs}")
                    nc.sync.dma_start(abt[:],
                                      attb_d[:, qs * XN:(qs + 1) * XN])

                for s in range(XN // NB):        # four z-windows per superstep
                    cs = slice(s * NB, (s + 1) * NB)
                    z = zp.tile([PP, NB], f32, tag="z", name=f"z_{qs}_{s}")
                    pe_anchor(z, cp)
                    # ISA caps one matmul at 512 columns: two half-window
                    # matmul groups fill the 1024-col PSUM tile
                    for h in range(NB // 512):
                        hs_ = slice(h * 512, (h + 1) * 512)
                        cs_h = slice(s * NB + h * 512, s * NB + (h + 1) * 512)
                        for c in range(2):
                            for b in range(B):
                                nc.tensor.matmul(z[b * MP:(b + 1) * MP, hs_],
                                                 wt[:, c * MP:(c + 1) * MP],
                                                 xq[(b, c)][:, cs_h],
                                                 start=(c == 0), stop=(c == 1))
                    k = qs * (XN // NB) + s
                    ys = slice(qs * XN + s * NB, qs * XN + (s + 1) * NB)
                    nc.vector.scalar_tensor_tensor(
                        out=y_full[:, ys], in0=z[:], scalar=1.0,
                        in1=abt[:, cs], op0=Alu.mult, op1=Alu.mult,
                        accum_out=s1t[:, k:k + 1])
                    sq = sqp.tile([PP, NB], bf16, tag="sq", name=f"sq_{qs}_{s}")
                    nc.scalar.activation(sq[:], y_full[:, ys], Act.Square,
                                         accum_out=s2t[:, k:k + 1])

            # ---- phase 2: reduce partials, AllReduce, BN fold ----
            from concourse.bass import _add_dep_helper
            prio = tc.high_priority()
            prio.__enter__()
            nc.vector.reduce_sum(st[:, 0:1], s1t[:], axis=mybir.AxisListType.X)
            nc.vector.reduce_sum(st[:, 1:2], s2t[:], axis=mybir.AxisListType.X)

            cc_in = dr.tile([PP, 2], f32)
            cc_out = dr.tile([PP, 2], f32)
            ccin_dma = nc.sync.dma_start(cc_in[:], st[:])
            nc.gpsimd.collective_compute(
                "AllReduce", Alu.add,
                replica_groups=[list(range(N_CORES))],
                ins=[cc_in[:].opt()],
                outs=[cc_out[:].opt()],
            )
            ar = sm.tile([PP, 2], f32, tag="ar")    # global sums
            ar_dma = nc.sync.dma_start(ar[:], cc_out[:])

            folded = stp.tile([M, 2], f32, tag="folded")
            nc.tensor.matmul(folded[:], foldWt, ar[:], start=True, stop=True)

            # foldW is pre-scaled by 1/NTOT on host: folded = (m, E[y^2])
            msq = sm.tile([M, 1], f32, tag="msq")
            nc.vector.tensor_scalar(msq[:], folded[:, 0:1], folded[:, 0:1],
                                    None, Alu.mult)
            vpe = sm.tile([M, 1], f32, tag="vpe")    # var + eps
            nc.vector.scalar_tensor_tensor(
                out=vpe[:], in0=folded[:, 1:2], scalar=EPS, in1=msq[:],
                op0=Alu.add, op1=Alu.subtract)
            sd = sm.tile([M, 1], f32, tag="sd")
            nc.scalar.activation(sd[:], vpe[:], Act.Sqrt)
            r = sm.tile([M, 1], f32, tag="r")
            nc.vector.reciprocal(r[:], sd[:])
            gh = sm.tile([M, 2], f32, tag="gh")      # (s', t') halved affine
            nc.vector.tensor_mul(gh[:, 0:1], r[:], gam)
            ms = sm.tile([M, 1], f32, tag="ms")
            nc.vector.tensor_scalar(ms[:], folded[:, 0:1], gh[:, 0:1],
                                    None, Alu.mult)
            nc.vector.tensor_sub(gh[:, 1:2], bet, ms[:])

            bc = stp.tile([PP, 2], f32, tag="bc")
            nc.tensor.matmul(bc[:], bcWt, gh[:], start=True, stop=True)
            stb = sm.tile([PP, 2], f32, tag="stb")
            nc.vector.tensor_copy(stb[:], bc[:])
            prio.__exit__(None, None, None)

            # ---- prefetch pnh: tile 0 right after the cc_in store (so the
            # store's HBM receipt — and thus the collective doorbell — is
            # not delayed), tile 1 after the AllReduce result lands so the
            # collective's HBM hops see a quiet memory system ----
            pnt = {}
            for qs in range(QS):
                t = pnl.tile([PP, XN], bf16, tag="pn", name=f"pn_{qs}")
                pdma = nc.sync.dma_start(t[:], pnh_d[:, qs * XN:(qs + 1) * XN])
                gate = ccin_dma if qs == 0 else ar_dma
                _add_dep_helper(pdma.ins, gate.ins, sync=True,
                                reason="stage pn prefetch around collective")
                pnt[qs] = t

            # ---- background-node path (independent; overlaps collective) ----
            pn0 = p0l.tile([128, 1280], bf16, tag="pn0")
            d1 = nc.sync.dma_start(pn0[:], p0h_d[:])
            hn0 = p0l.tile([128, 1280], bf16, tag="hn0")
            d2 = nc.sync.dma_start(hn0[:], h0h_d[:])
            _add_dep_helper(d1.ins, ccin_dma.ins, sync=True,
                            reason="defer p0 loads past cc_in store")
            _add_dep_helper(d2.ins, ccin_dma.ins, sync=True,
                            reason="defer p0 loads past cc_in store")
            o0 = p0l.tile([128, 1280], bf16, tag="o0")
            nc.vector.tensor_add(o0[:], pn0[:], hn0[:])
            nc.sync.dma_start(out0_d[:], o0[:])

            # ---- phase 3: d = relu(s'*y + t') ; out = pnh + d.
            # Most windows: ACT does the fused relu-affine, DVE adds pnh.
            # One window per superstep runs entirely on DVE (tensor_scalar
            # affine + stt relu-add) to balance the two engines. ----
            for qs in range(QS):
                for s in range(XN // NQ):
                    ys = slice(qs * XN + s * NQ, qs * XN + (s + 1) * NQ)
                    ps = slice(s * NQ, (s + 1) * NQ)
                    o = obuf.tile([PP, NQ], bf16, tag="o", bufs=3,
                                  name=f"o_{qs}_{s}")
                    if s == 1:                           # DVE-only window
                        t1 = obuf.tile([PP, NQ], bf16, tag="d", bufs=3,
                                       name=f"t1_{qs}_{s}")
                        nc.vector.tensor_scalar(
                            t1[:], y_full[:, ys], stb[:, 0:1], stb[:, 1:2],
                            Alu.mult, Alu.add)
                        nc.vector.scalar_tensor_tensor(
                            out=o[:], in0=t1[:], scalar=0.0,
                            in1=pnt[qs][:, ps], op0=Alu.max, op1=Alu.add)
                    else:
                        d = obuf.tile([PP, NQ], bf16, tag="d", bufs=3,
                                      name=f"d_{qs}_{s}")
                        nc.scalar.activation(d[:], y_full[:, ys], Act.Relu,
                                             scale=stb[:, 0:1],
                                             bias=stb[:, 1:2])
                        nc.vector.tensor_add(o[:], pnt[qs][:, ps], d[:])
                    nc.sync.dma_start(out_d[:, ys], o[:])

    # hoist excess sync waits onto same-engine NOPs (walrus wait-slot limits)
    SI = bass_rust.SyncInfo
    k = 0
    for fn in nc.m.functions:
        for bb in fn.blocks:
            out = []
            for ins in bb.instructions:
                si = ins.sync_info
                if si is not None and len(si.on_wait) > 1:
                    waits = list(si.on_wait)
                    extra, keep = waits[:-1], waits[-1:]
                    for wti in extra:
                        nop = bass_rust.InstNoOp(name=f"Wsplit-{k}", ins=[], outs=[])
                        k += 1
                        nop.engine = ins.engine
                        nop.sync_info = SI(on_wait=[wti], on_update=[])
                        out.append(nop)
                    ins.sync_info = SI(on_wait=keep, on_update=list(si.on_update))
                out.append(ins)
            bb.instructions = out
    return nc


def _get_nc():
    global _built
    if _built is None:
        _built = _build()
    return _built


def _prep_core(i, p_nodes_h, h_nodes0_h, xp, h_att, cpack, wtb):
    hs = i * HS
    xp_t = np.ascontiguousarray(
        xp[:, :, hs:hs + HS, :].transpose(1, 0, 2, 3)).reshape(C, B * SPB)
    xp_t = xp_t.astype(BF16)
    attb = np.zeros((PP, SPB), BF16)
    for b in range(B):
        attb[b * MP:b * MP + 40] = h_att[1, b, 0, hs:hs + HS, :].reshape(
            1, SPB).astype(BF16)
        attb[b * MP + 40:b * MP + 60] = h_att[2, b, 0, hs:hs + HS, :].reshape(
            1, SPB).astype(BF16)
    pn16 = p_nodes_h[1:7, :, :, hs:hs + HS, :]          # halved [6,B,10,HS,W]
    pn16 = pn16.transpose(1, 0, 2, 3, 4).reshape(B, M, SPB)
    pnh = np.zeros((PP, SPB), BF16)
    pnh[0:M] = pn16[0]
    pnh[MP:MP + M] = pn16[1]
    p0h = np.ascontiguousarray(
        p_nodes_h[0, :, :, hs:hs + HS, :]).reshape(128, 1280)
    h0h = np.ascontiguousarray(
        h_nodes0_h[:, :, hs:hs + HS, :]).reshape(128, 1280)
    return {"xp": xp_t, "attb": attb, "pnh": pnh,
            "p0h": p0h, "h0h": h0h, "cpack": cpack, "wtb": wtb}


def _make_consts(Wu, Wl, gamma_u, beta_u, gamma_l, beta_l):
    f32 = np.float32
    Wcat = np.concatenate([Wu, Wl], 0)                # [60, 256]
    lhsT = np.zeros((C, MP), f32)
    lhsT[:, 0:M] = Wcat.T
    wtb = np.zeros((128, 256), BF16)
    wtb[:, 0:MP] = lhsT[0:128].astype(BF16)
    wtb[:, MP:2 * MP] = lhsT[128:256].astype(BF16)
    # att-broadcast lhsT: L4t[k, p] = 1 iff channel partition p uses
    # attention row k (rows: a1b0, a2b0, a1b1, a2b1)
    L4 = np.zeros((4, 128), BF16)
    L4[0, 0:40] = 1; L4[1, 40:60] = 1
    L4[2, MP:MP + 40] = 1; L4[3, MP + 40:MP + 60] = 1
    wtb[0:4, 128:256] = L4
    cpack = np.zeros((128, CW), f32)
    foldW = np.zeros((PP, M), f32)
    foldW[0:M] = np.eye(M, dtype=f32) / NTOT
    foldW[MP:MP + M] = np.eye(M, dtype=f32) / NTOT
    cpack[0:PP, C_FOLD:C_FOLD + M] = foldW
    bcW = np.zeros((M, PP), f32)
    bcW[:, 0:M] = np.eye(M, dtype=f32)
    bcW[:, MP:MP + M] = np.eye(M, dtype=f32)
    cpack[0:M, C_BC:C_BC + PP] = bcW
    cpack[0:M, C_GB] = 0.5 * np.concatenate([gamma_u, gamma_l])
    cpack[0:M, C_GB + 1] = 0.5 * np.concatenate([beta_u, beta_l])
    return cpack, wtb


def _run(inputs, trace=False, trace_cores=None):
    from concourse import bass_utils
    p_nodes = np.asarray(inputs["p_nodes"], np.float32)
    h_nodes = np.asarray(inputs["h_nodes"], np.float32)
    xp = np.asarray(inputs["xp"], np.float32)
    h_att = np.asarray(inputs["h_att"], np.float32)
    cpack, wtb = _make_consts(np.asarray(inputs["Wu"], np.float32),
                              np.asarray(inputs["Wl"], np.float32),
                              np.asarray(inputs["gamma_u"], np.float32),
                              np.asarray(inputs["beta_u"], np.float32),
                              np.asarray(inputs["gamma_l"], np.float32),
                              np.asarray(inputs["beta_l"], np.float32))
    p_nodes_h = (0.5 * p_nodes).astype(BF16)
    h_nodes0_h = (0.5 * h_nodes[0]).astype(BF16)
    in_maps = [_prep_core(i, p_nodes_h, h_nodes0_h, xp, h_att, cpack, wtb)
               for i in range(N_CORES)]
    nc = _get_nc()
    res = bass_utils.run_bass_kernel_spmd(
        nc, in_maps, core_ids=list(range(N_CORES)), trace=trace,
        trace_cores=trace_cores)

    p_new = np.empty((7, B, HID, H, W), np.float32)
    for i in range(N_CORES):
        hs = i * HS
        om = res.results[i]["out_main"]               # [128, SPB] bf16
        o0 = res.results[i]["out0"]                   # [128, 1280] bf16
        p_new[0, :, :, hs:hs + HS, :] = o0.astype(np.float32).reshape(
            B, HID, HS, W)
        for b in range(B):
            blk = om[b * MP:b * MP + M].astype(np.float32).reshape(
                6, HID, HS, W)
            p_new[1:7, b, :, hs:hs + HS, :] = blk
    return p_new, res


def kernel(**inputs) -> np.ndarray:
    return _run(inputs, trace=False)[0]


# revision 54
# speedup vs baseline: 1.0536x; 1.0536x over previous
"""Trainium2 Bass kernel for nn_GNN_82781199663565 (gnn_message_passing).

Computation (see reference):
  du = relu(BN(einsum(h_att[1]*xp, Wu)))   # [B, 40, H, W]
  dl = relu(BN(einsum(h_att[2]*xp, Wl)))   # [B, 20, H, W]
  p_new[0]   = 0.5*(h_nodes[0] + p_nodes[0])
  p_new[1:5] = 0.5*(p_nodes[1:5] + du4)    # du reshaped to [4, B, 10, H, W]
  p_new[5:7] = 0.5*(p_nodes[5:7] + dl2)
(f_nodes, h_att[0], h_nodes[1:] are unused.)

Strategy v2: data-parallel over H (32 rows per core, 8 cores), bf16 streams.
 - All big tensors (xp, attention, p_nodes residuals, outputs) travel as
   bf16: halves HBM traffic and runs the PE at full bf16 rate (fp32 matmul
   is ~4x slower).  End-to-end rel-err ~4e-3 (gate 2e-2).
 - One fused matmul z = Wcat.T @ xp per 512-col window, both batch images
   stacked on partitions (b0 -> 0:64, b1 -> 64:128).
 - Attention applied after the conv via a host-replicated bf16 [128, n]
   array; one DVE op computes y = z*a and accumulates sum(y); ACT squares
   for sum(y^2).
 - Sync-BN stats via ncfw AllReduce (8 cores, 1 KB).  No warm-up collective
   (a warm-up queues ahead on the FIFO CC stream and delays the real op);
   the pn/p0 prefetches are ordered after the cc_in store completes so HBM
   contention cannot delay the collective doorbell.  (A hand-rolled SWDGE
   remote-DMA mailbox was tried and reverted: under this runtime the remote
   sends take ~4 ms to deliver.)
 - p_nodes / h_nodes residuals are pre-halved host-side; phase 3 is
   d = relu(s'*y + t') on ACT and out = pnh + d on DVE, stores in bf16.
"""
import sys
sys.path.insert(0, '/opt/trn_rl_repo')

import numpy as np
import ml_dtypes

BF16 = ml_dtypes.bfloat16

N_CORES = 8
B, C, HID, H, W = 2, 256, 10, 256, 256
EPS = 1e-5
HS = H // N_CORES            # 32 H-rows per core
SPB = HS * W                 # spatial elems per batch image per core: 8192
M = 60                       # real output channels (40 u + 20 l)
MP = 64                      # padded to 64 -> groups tile partitions exactly
PP = 128
NB = 1024                    # matmul block (bf16 rhs max; 2 PSUM banks)
NQ = 2048                    # phase-3 window
NTOT = float(B * H * W)      # BN stat count: 131072
XN = 4096                    # superstep spatial columns (1 MiB bf16 loads)
QS = SPB // XN               # 2 supersteps

# packed fp32 constants column offsets: foldW, bcW, gamma, beta
C_FOLD = 0
C_BC = C_FOLD + M
C_GB = C_BC + PP
CW = C_GB + 2

_built = None


def _build():
    import concourse.bass as bass
    import concourse.tile as tile
    from concourse import mybir
    import bass_rust

    f32 = mybir.dt.float32
    bf16 = mybir.dt.bfloat16
    Alu = mybir.AluOpType
    Act = mybir.ActivationFunctionType

    nc = bass.Bass("TRN2", target_bir_lowering=False, debug=False,
                   num_devices=N_CORES, enable_partition_id=False)

    xp_d = nc.dram_tensor("xp", [C, B * SPB], bf16, kind="ExternalInput").ap()
    attb_d = nc.dram_tensor("attb", [PP, SPB], bf16, kind="ExternalInput").ap()
    pnh_d = nc.dram_tensor("pnh", [PP, SPB], bf16, kind="ExternalInput").ap()
    p0h_d = nc.dram_tensor("p0h", [128, 1280], bf16, kind="ExternalInput").ap()
    h0h_d = nc.dram_tensor("h0h", [128, 1280], bf16, kind="ExternalInput").ap()
    cpack_d = nc.dram_tensor("cpack", [128, CW], f32, kind="ExternalInput").ap()
    wtb_d = nc.dram_tensor("wtb", [128, 256], bf16, kind="ExternalInput").ap()

    out_d = nc.dram_tensor("out_main", [PP, SPB], bf16, kind="ExternalOutput").ap()
    out0_d = nc.dram_tensor("out0", [128, 1280], bf16, kind="ExternalOutput").ap()

    def pe_anchor(psum_tile, cp):
        # tiny matmul reading cp (seen by PE) writing one psum element:
        # absorbs the psum slot-release wait so real matmuls carry <=1 wait
        nc.tensor.matmul(psum_tile[0:1, 0:1], cp[0:1, 0:1], cp[0:1, 0:1],
                         start=True, stop=True, skip_group_check=True)

    with tile.TileContext(nc) as tc:
        with (
            tc.tile_pool(name="consts", bufs=1) as cpool,
            tc.tile_pool(name="attp", bufs=2) as attp,
            tc.tile_pool(name="xin", bufs=2) as xin,
            tc.tile_pool(name="ybuf", bufs=1) as ybuf,
            tc.tile_pool(name="sq", bufs=2) as sqp,
            tc.tile_pool(name="small", bufs=1) as sm,
            tc.tile_pool(name="pnl", bufs=2) as pnl,
            tc.tile_pool(name="p0l", bufs=1) as p0l,
            tc.tile_pool(name="obuf", bufs=2) as obuf,
            tc.tile_pool(name="zp", bufs=3, space="PSUM") as zp,
            tc.tile_pool(name="stp", bufs=1, space="PSUM") as stp,
            tc.tile_pool(name="dram", bufs=1, space="DRAM") as dr,
        ):
            # issue the first superstep's xp loads before anything else so
            # the HBM stream starts the moment the preamble ends
            xq0 = {}
            for b in range(B):
                for c in range(2):
                    t = xin.tile([128, XN], bf16, tag=f"x{b}{c}",
                                 name=f"x{b}{c}_0")
                    lo = b * SPB
                    nc.sync.dma_start(
                        t[:], xp_d[c * 128:(c + 1) * 128, lo:lo + XN])
                    xq0[(b, c)] = t
            abt0 = attp.tile([PP, XN], bf16, tag="attb", name="attb_0")
            nc.sync.dma_start(abt0[:], attb_d[:, 0:XN])

            cp = cpool.tile([128, CW], f32)
            nc.sync.dma_start(cp[:], cpack_d[:])
            wt = cpool.tile([128, 256], bf16, tag="wt")
            nc.sync.dma_start(wt[:], wtb_d[:])
            L4t = wt[0:4, 128:256]          # att-broadcast lhsT
            foldWt = cp[0:PP, C_FOLD:C_FOLD + M]
            bcWt = cp[0:M, C_BC:C_BC + PP]
            gam = cp[0:M, C_GB:C_GB + 1]      # 0.5*gamma (u|l)
            bet = cp[0:M, C_GB + 1:C_GB + 2]  # 0.5*beta

            y_full = ybuf.tile([PP, SPB], bf16)
            s1t = sm.tile([PP, (SPB // NB)], f32, tag="s1t")
            s2t = sm.tile([PP, (SPB // NB)], f32, tag="s2t")
            st = sm.tile([PP, 2], f32, tag="st")     # local BN partial sums

            # ---- PE warm-up: bf16 dummy matmuls trip the HAM toward the
            # 2.4 GHz state before the first xp tile lands ----
            wz = zp.tile([PP, NB], f32, tag="z", name="warm_z")
            for _ in range(16):
                nc.tensor.matmul(wz[0:128, 0:128], wt[:, 0:128], wt[:, 0:128],
                                 start=True, stop=True, skip_group_check=True)

            # ---- phase 1: stream xp, matmul, y = z*a, accumulate sums ----
            for qs in range(QS):
                if qs == 0:
                    xq, abt = xq0, abt0
                else:
                    # split loads into halves, first halves of every tile
                    # first: the early windows of this superstep can start
                    # while the second halves still stream, so compute is
                    # not bunched after the stream ends
                    xq = {}
                    for b in range(B):
                        for c in range(2):
                            xq[(b, c)] = xin.tile(
                                [128, XN], bf16, tag=f"x{b}{c}",
                                name=f"x{b}{c}_{qs}")
                    abt = attp.tile([PP, XN], bf16, tag="attb",
                                    name=f"attb_{qs}")
                    for h2 in range(2):
                        cl = slice(h2 * XN // 2, (h2 + 1) * XN // 2)
                        for b in range(B):
                            for c in range(2):
                                lo = b * SPB + qs * XN + h2 * XN // 2
                                xdma = nc.sync.dma_start(
                                    xq[(b, c)][:, cl],
                                    xp_d[c * 128:(c + 1) * 128,
                                         lo:lo + XN // 2])
                                if h2 == 1 and b == B - 1 and c == 1:
                                    last_xdma = xdma
                        nc.sync.dma_start(
                            abt[:, cl],
                            attb_d[:, qs * XN + h2 * XN // 2:
                                   qs * XN + (h2 + 1) * XN // 2])

                for s in range(XN // NB):        # four z-windows per superstep
                    cs = slice(s * NB, (s + 1) * NB)
                    z = zp.tile([PP, NB], f32, tag="z", name=f"z_{qs}_{s}")
                    pe_anchor(z, cp)
                    # ISA caps one matmul at 512 columns: two half-window
                    # matmul groups fill the 1024-col PSUM tile
                    for h in range(NB // 512):
                        hs_ = slice(h * 512, (h + 1) * 512)
                        cs_h = slice(s * NB + h * 512, s * NB + (h + 1) * 512)
                        for c in range(2):
                            for b in range(B):
                                nc.tensor.matmul(z[b * MP:(b + 1) * MP, hs_],
                                                 wt[:, c * MP:(c + 1) * MP],
                                                 xq[(b, c)][:, cs_h],
                                                 start=(c == 0), stop=(c == 1))
                    k = qs * (XN // NB) + s
                    ys = slice(qs * XN + s * NB, qs * XN + (s + 1) * NB)
                    nc.vector.scalar_tensor_tensor(
                        out=y_full[:, ys], in0=z[:], scalar=1.0,
                        in1=abt[:, cs], op0=Alu.mult, op1=Alu.mult,
                        accum_out=s1t[:, k:k + 1])
                    sq = sqp.tile([PP, NB], bf16, tag="sq", name=f"sq_{qs}_{s}")
                    if qs == QS - 1 and s == XN // NB - 1:
                        # last window: square on DVE so the stats reduce is
                        # not serialized behind a trailing ACT op
                        nc.vector.scalar_tensor_tensor(
                            out=sq[:], in0=y_full[:, ys], scalar=1.0,
                            in1=y_full[:, ys], op0=Alu.mult, op1=Alu.mult,
                            accum_out=s2t[:, k:k + 1])
                    else:
                        nc.scalar.activation(sq[:], y_full[:, ys], Act.Square,
                                             accum_out=s2t[:, k:k + 1])

            # ---- phase 2: reduce partials, AllReduce, BN fold ----
            from concourse.bass import _add_dep_helper
            prio = tc.high_priority()
            prio.__enter__()
            nc.vector.reduce_sum(st[:, 0:1], s1t[:], axis=mybir.AxisListType.X)
            nc.vector.reduce_sum(st[:, 1:2], s2t[:], axis=mybir.AxisListType.X)

            cc_in = dr.tile([PP, 2], f32)
            cc_out = dr.tile([PP, 2], f32)
            ccin_dma = nc.sync.dma_start(cc_in[:], st[:])
            nc.gpsimd.collective_compute(
                "AllReduce", Alu.add,
                replica_groups=[list(range(N_CORES))],
                ins=[cc_in[:].opt()],
                outs=[cc_out[:].opt()],
            )
            ar = sm.tile([PP, 2], f32, tag="ar")    # global sums
            ar_dma = nc.sync.dma_start(ar[:], cc_out[:])

            folded = stp.tile([M, 2], f32, tag="folded")
            nc.tensor.matmul(folded[:], foldWt, ar[:], start=True, stop=True)

            # foldW is pre-scaled by 1/NTOT on host: folded = (m, E[y^2])
            msq = sm.tile([M, 1], f32, tag="msq")
            nc.vector.tensor_scalar(msq[:], folded[:, 0:1], folded[:, 0:1],
                                    None, Alu.mult)
            vpe = sm.tile([M, 1], f32, tag="vpe")    # var + eps
            nc.vector.scalar_tensor_tensor(
                out=vpe[:], in0=folded[:, 1:2], scalar=EPS, in1=msq[:],
                op0=Alu.add, op1=Alu.subtract)
            sd = sm.tile([M, 1], f32, tag="sd")
            nc.scalar.activation(sd[:], vpe[:], Act.Sqrt)
            r = sm.tile([M, 1], f32, tag="r")
            nc.vector.reciprocal(r[:], sd[:])
            gh = sm.tile([M, 2], f32, tag="gh")      # (s', t') halved affine
            nc.vector.tensor_mul(gh[:, 0:1], r[:], gam)
            ms = sm.tile([M, 1], f32, tag="ms")
            nc.vector.tensor_scalar(ms[:], folded[:, 0:1], gh[:, 0:1],
                                    None, Alu.mult)
            nc.vector.tensor_sub(gh[:, 1:2], bet, ms[:])

            bc = stp.tile([PP, 2], f32, tag="bc")
            nc.tensor.matmul(bc[:], bcWt, gh[:], start=True, stop=True)
            stb = sm.tile([PP, 2], f32, tag="stb")
            nc.vector.tensor_copy(stb[:], bc[:])
            prio.__exit__(None, None, None)

            # ---- prefetch pnh: tile 0 right after the cc_in store (so the
            # store's HBM receipt — and thus the collective doorbell — is
            # not delayed), tile 1 after the AllReduce result lands so the
            # collective's HBM hops see a quiet memory system ----
            pnt = {}
            for qs in range(QS):
                t = pnl.tile([PP, XN], bf16, tag="pn", name=f"pn_{qs}")
                pdma = nc.sync.dma_start(t[:], pnh_d[:, qs * XN:(qs + 1) * XN])
                gate = ccin_dma if qs == 0 else ar_dma
                _add_dep_helper(pdma.ins, gate.ins, sync=True,
                                reason="stage pn prefetch around collective")
                pnt[qs] = t

            # ---- background-node path (independent; overlaps collective) ----
            pn0 = p0l.tile([128, 1280], bf16, tag="pn0")
            d1 = nc.sync.dma_start(pn0[:], p0h_d[:])
            hn0 = p0l.tile([128, 1280], bf16, tag="hn0")
            d2 = nc.sync.dma_start(hn0[:], h0h_d[:])
            _add_dep_helper(d1.ins, ccin_dma.ins, sync=True,
                            reason="defer p0 loads past cc_in store")
            _add_dep_helper(d2.ins, ccin_dma.ins, sync=True,
                            reason="defer p0 loads past cc_in store")
            o0 = p0l.tile([128, 1280], bf16, tag="o0")
            nc.vector.tensor_add(o0[:], pn0[:], hn0[:])
            nc.sync.dma_start(out0_d[:], o0[:])

            # ---- phase 3: d = relu(s'*y + t') ; out = pnh + d.
            # Most windows: ACT does the fused relu-affine, DVE adds pnh.
            # One window per superstep runs entirely on DVE (tensor_scalar
            # affine + stt relu-add) to balance the two engines. ----
            for qs in range(QS):
                for s in range(XN // NQ):
                    ys = slice(qs * XN + s * NQ, qs * XN + (s + 1) * NQ)
                    ps = slice(s * NQ, (s + 1) * NQ)
                    o = obuf.tile([PP, NQ], bf16, tag="o", bufs=3,
                                  name=f"o_{qs}_{s}")
                    if s == 1:                           # DVE-only window
                        t1 = obuf.tile([PP, NQ], bf16, tag="d", bufs=3,
                                       name=f"t1_{qs}_{s}")
                        nc.vector.tensor_scalar(
                            t1[:], y_full[:, ys], stb[:, 0:1], stb[:, 1:2],
                            Alu.mult, Alu.add)
                        nc.vector.scalar_tensor_tensor(
                            out=o[:], in0=t1[:], scalar=0.0,
                            in1=pnt[qs][:, ps], op0=Alu.max, op1=Alu.add)
                    else:
                        d = obuf.tile([PP, NQ], bf16, tag="d", bufs=3,
                                      name=f"d_{qs}_{s}")
                        nc.scalar.activation(d[:], y_full[:, ys], Act.Relu,
                                             scale=stb[:, 0:1],
                                             bias=stb[:, 1:2])
                        nc.vector.tensor_add(o[:], pnt[qs][:, ps], d[:])
                    nc.sync.dma_start(out_d[:, ys], o[:])

    # hoist excess sync waits onto same-engine NOPs (walrus wait-slot limits)
    SI = bass_rust.SyncInfo
    k = 0
    for fn in nc.m.functions:
        for bb in fn.blocks:
            out = []
            for ins in bb.instructions:
                si = ins.sync_info
                if si is not None and len(si.on_wait) > 1:
                    waits = list(si.on_wait)
                    extra, keep = waits[:-1], waits[-1:]
                    for wti in extra:
                        nop = bass_rust.InstNoOp(name=f"Wsplit-{k}", ins=[], outs=[])
                        k += 1
                        nop.engine = ins.engine
                        nop.sync_info = SI(on_wait=[wti], on_update=[])
                        out.append(nop)
                    ins.sync_info = SI(on_wait=keep, on_update=list(si.on_update))
                out.append(ins)
            bb.instructions = out
    return nc


def _get_nc():
    global _built
    if _built is None:
        _built = _build()
    return _built


def _prep_core(i, p_nodes_h, h_nodes0_h, xp, h_att, cpack, wtb):
    hs = i * HS
    xp_t = np.ascontiguousarray(
        xp[:, :, hs:hs + HS, :].transpose(1, 0, 2, 3)).reshape(C, B * SPB)
    xp_t = xp_t.astype(BF16)
    attb = np.zeros((PP, SPB), BF16)
    for b in range(B):
        attb[b * MP:b * MP + 40] = h_att[1, b, 0, hs:hs + HS, :].reshape(
            1, SPB).astype(BF16)
        attb[b * MP + 40:b * MP + 60] = h_att[2, b, 0, hs:hs + HS, :].reshape(
            1, SPB).astype(BF16)
    pn16 = p_nodes_h[1:7, :, :, hs:hs + HS, :]          # halved [6,B,10,HS,W]
    pn16 = pn16.transpose(1, 0, 2, 3, 4).reshape(B, M, SPB)
    pnh = np.zeros((PP, SPB), BF16)
    pnh[0:M] = pn16[0]
    pnh[MP:MP + M] = pn16[1]
    p0h = np.ascontiguousarray(
        p_nodes_h[0, :, :, hs:hs + HS, :]).reshape(128, 1280)
    h0h = np.ascontiguousarray(
        h_nodes0_h[:, :, hs:hs + HS, :]).reshape(128, 1280)
    return {"xp": xp_t, "attb": attb, "pnh": pnh,
            "p0h": p0h, "h0h": h0h, "cpack": cpack, "wtb": wtb}


def _make_consts(Wu, Wl, gamma_u, beta_u, gamma_l, beta_l):
    f32 = np.float32
    Wcat = np.concatenate([Wu, Wl], 0)                # [60, 256]
    lhsT = np.zeros((C, MP), f32)
    lhsT[:, 0:M] = Wcat.T
    wtb = np.zeros((128, 256), BF16)
    wtb[:, 0:MP] = lhsT[0:128].astype(BF16)
    wtb[:, MP:2 * MP] = lhsT[128:256].astype(BF16)
    # att-broadcast lhsT: L4t[k, p] = 1 iff channel partition p uses
    # attention row k (rows: a1b0, a2b0, a1b1, a2b1)
    L4 = np.zeros((4, 128), BF16)
    L4[0, 0:40] = 1; L4[1, 40:60] = 1
    L4[2, MP:MP + 40] = 1; L4[3, MP + 40:MP + 60] = 1
    wtb[0:4, 128:256] = L4
    cpack = np.zeros((128, CW), f32)
    foldW = np.zeros((PP, M), f32)
    foldW[0:M] = np.eye(M, dtype=f32) / NTOT
    foldW[MP:MP + M] = np.eye(M, dtype=f32) / NTOT
    cpack[0:PP, C_FOLD:C_FOLD + M] = foldW
    bcW = np.zeros((M, PP), f32)
    bcW[:, 0:M] = np.eye(M, dtype=f32)
    bcW[:, MP:MP + M] = np.eye(M, dtype=f32)
    cpack[0:M, C_BC:C_BC + PP] = bcW
    cpack[0:M, C_GB] = 0.5 * np.concatenate([gamma_u, gamma_l])
    cpack[0:M, C_GB + 1] = 0.5 * np.concatenate([beta_u, beta_l])
    return cpack, wtb


def _run(inputs, trace=False, trace_cores=None):
    from concourse import bass_utils
    p_nodes = np.asarray(inputs["p_nodes"], np.float32)
    h_nodes = np.asarray(inputs["h_nodes"], np.float32)
    xp = np.asarray(inputs["xp"], np.float32)
    h_att = np.asarray(inputs["h_att"], np.float32)
    cpack, wtb = _make_consts(np.asarray(inputs["Wu"], np.float32),
                              np.asarray(inputs["Wl"], np.float32),
                              np.asarray(inputs["gamma_u"], np.float32),
                              np.asarray(inputs["beta_u"], np.float32),
                              np.asarray(inputs["gamma_l"], np.float32),
                              np.asarray(inputs["beta_l"], np.float32))
    p_nodes_h = (0.5 * p_nodes).astype(BF16)
    h_nodes0_h = (0.5 * h_nodes[0]).astype(BF16)
    in_maps = [_prep_core(i, p_nodes_h, h_nodes0_h, xp, h_att, cpack, wtb)
               for i in range(N_CORES)]
    nc = _get_nc()
    res = bass_utils.run_bass_kernel_spmd(
        nc, in_maps, core_ids=list(range(N_CORES)), trace=trace,
        trace_cores=trace_cores)

    p_new = np.empty((7, B, HID, H, W), np.float32)
    for i in range(N_CORES):
        hs = i * HS
        om = res.results[i]["out_main"]               # [128, SPB] bf16
        o0 = res.results[i]["out0"]                   # [128, 1280] bf16
        p_new[0, :, :, hs:hs + HS, :] = o0.astype(np.float32).reshape(
            B, HID, HS, W)
        for b in range(B):
            blk = om[b * MP:b * MP + M].astype(np.float32).reshape(
                6, HID, HS, W)
            p_new[1:7, b, :, hs:hs + HS, :] = blk
    return p_new, res


def kernel(**inputs) -> np.ndarray:
    return _run(inputs, trace=False)[0]
